# revision 18
# baseline (speedup 1.0000x reference)
"""Causal multi-head attention on 8 trn2 NeuronCores.

Sharding: core c handles batch b=c//4 and heads [4*(c%4), 4*(c%4)+4).
Each core computes its 4 heads' attention plus the partial output
projection against the matching 256 rows of Wo; the host sums the 4
partials per batch (the all-reduce implied by row-sharding Wo) and adds
bo_eff = bo + bv_cat @ Wo (the V-bias folds out of the kernel because
softmax rows sum to one).

v2 layout strategy (bf16 matmul operands, fp32 PSUM accumulation):
  - X^T [D,S] bf16 in SBUF; every projection contracts d on partitions.
  - Q^T/K^T per head-pair [128, S] bf16 (two heads stacked on
    partitions). Weight-stationary loop order: one LDWEIGHTS per
    (pair, d-chunk) feeds 4 query-block matmuls.
  - Scores transposed ST[kv, q] = K^T.T @ Q^T, head pair ROW-TILED:
    even head contracts on partitions 0:64 (tile_position (0,0)), odd
    on 64:128 ((64,0)) -> the two matmuls run concurrently in the PE
    array, writing adjacent PSUM banks of one [128, 2, 512] tile.
  - One exp per (pair, chunk) over both heads' scores (scale=1/8
    applied inside the activation); output pt bf16. Causal masking is
    a post-exp elementwise multiply with a 0/1 triangle on DVE -
    no mask matmuls, no -1e9 constants.
  - ctx^T via Vaug trick (ones column carries the softmax denominator
    into a spare PSUM partition). Normalization: reciprocal_approx_fast
    (~5x faster than DVE reciprocal), rank-1 PE broadcast, one DVE
    multiply straight into bf16 ctxcat. No bias add (folded into host).
  - Output projection bf16, PSUM->SBUF eviction on DVE, bf16 DMA out.
"""

import sys

for _p in ("/opt/trn_rl_repo", "/root/.axon_site/_ro/trn_rl_repo"):
    if _p not in sys.path:
        sys.path.insert(0, _p)

import numpy as np

import concourse.bass as bass
import concourse.bacc as bacc
import concourse.tile as tile
from concourse import mybir
from concourse.bass_utils import run_bass_kernel_spmd

F32 = mybir.dt.float32
BF16 = mybir.dt.bfloat16

B, S, D, H, DK = 2, 2048, 1024, 16, 64
NCORES = 8
HPC = 4          # heads per core
NPAIR = 2        # head pairs per core
ND = D // 128    # 8 contraction chunks over d
NS = S // 512    # 4 query blocks
NS16 = S // 128  # 16 sequence chunks

_CACHE = {}


def _build_bass():
    nc = bacc.Bacc(None)
    xt = nc.dram_tensor("xt", [128, ND, S], BF16, kind="ExternalInput")
    wq = nc.dram_tensor("wq", [128, NPAIR, ND, 128], BF16, kind="ExternalInput")
    wk = nc.dram_tensor("wk", [128, NPAIR, ND, 128], BF16, kind="ExternalInput")
    wv = nc.dram_tensor("wv", [128, ND, 256], BF16, kind="ExternalInput")
    wo = nc.dram_tensor("wo", [128, 2, D], BF16, kind="ExternalInput")
    bq = nc.dram_tensor("bq", [128, NPAIR], F32, kind="ExternalInput")
    bk = nc.dram_tensor("bk", [128, NPAIR], F32, kind="ExternalInput")
    mask01 = nc.dram_tensor("mask01", [128, 128], BF16, kind="ExternalInput")
    bcsel = nc.dram_tensor("bcsel", [128, 128], BF16, kind="ExternalInput")
    onescol = nc.dram_tensor("onescol", [128, 8], BF16, kind="ExternalInput")
    out = nc.dram_tensor("out", [S, D], BF16, kind="ExternalOutput")

    with nc.allow_low_precision("bf16 operands; accumulation stays fp32 in PSUM"), \
            tile.TileContext(nc) as tc:
        with (
            tc.tile_pool(name="consts", bufs=1) as consts,
            tc.tile_pool(name="qkv", bufs=1) as qkv,
        ):
            wq_sb = consts.tile([128, NPAIR, ND, 128], BF16, tag="wq")
            wk_sb = consts.tile([128, NPAIR, ND, 128], BF16, tag="wk")
            wv_sb = consts.tile([128, ND, 256], BF16, tag="wv")
            wo_sb = consts.tile([128, 2, D], BF16, tag="wo")
            bq_sb = consts.tile([128, NPAIR], F32, tag="bq")
            bk_sb = consts.tile([128, NPAIR], F32, tag="bk")
            mask01_sb = consts.tile([128, 128], BF16, tag="mask01")
            bcsel_sb = consts.tile([128, 128], BF16, tag="bcsel")
            onescol_sb = consts.tile([128, 8], BF16, tag="onescol")

            qt_sb = qkv.tile([128, NPAIR, S], BF16, tag="qt")
            kt_sb = qkv.tile([128, NPAIR, S], BF16, tag="kt")
            # V per pair: cols 0:64 V_even | 64:128 zeros | 128:192
            # V_odd. ctx_e uses cols 0:64 -> parts 0:64 of bank A.
            # ctx_o: at c==0, cols 64:192 (M=128, start=True) writes the
            # whole bank B (zeros to parts 0:64, ctx to 64:128) so the
            # accumulate bits are established for every partition; for
            # c>0 it uses cols 128:192 with col-tile (0,64). Softmax
            # denominators are rank-1 matmuls into bank B parts 0 (odd)
            # and 32 (even) at col groups 0/1, so for c>0 the ctx pair
            # and the denom pair each run concurrently in the PE array.
            va_sb = qkv.tile([128, NPAIR, NS16, 192], BF16, tag="va")
            ctxcat_sb = qkv.tile([128, 2, S], BF16, tag="ctxcat")

            # small consts first (tiny transfers, needed early)
            actwarm = consts.tile([1, 2], F32, tag="actwarm")
            nc.sync.dma_start(out=bq_sb[:], in_=bq[:])
            # preload the exp table set while DMAs stream (one-time
            # ACT_TABLE_LOAD ~2.7us otherwise lands mid-attention)
            nc.scalar.activation(
                out=actwarm[:], in_=bq_sb[0:1, 0:2],
                func=mybir.ActivationFunctionType.Exp,
            )
            nc.sync.dma_start(out=bk_sb[:], in_=bk[:])
            nc.sync.dma_start(out=mask01_sb[:], in_=mask01[:])
            nc.sync.dma_start(out=bcsel_sb[:], in_=bcsel[:])
            nc.sync.dma_start(out=onescol_sb[:], in_=onescol[:])
            for p in range(NPAIR):
                nc.vector.memset(va_sb[:, p, :, 64:128], 0.0)

            with (
                tc.tile_pool(name="xp", bufs=1) as xp,
            ):
                xt_sb = xp.tile([128, ND, S], BF16, tag="xt")
                nc.gpsimd.dma_start(out=wq_sb[:, 0], in_=wq[:, 0])
                nc.scalar.dma_start(out=wk_sb[:, 0], in_=wk[:, 0])
                nc.gpsimd.dma_start(out=wq_sb[:, 1], in_=wq[:, 1])
                nc.scalar.dma_start(out=wk_sb[:, 1], in_=wk[:, 1])
                nc.gpsimd.dma_start(out=wv_sb[:], in_=wv[:])
                nc.scalar.dma_start(out=wo_sb[:], in_=wo[:])
                for c in range(ND):
                    nc.sync.dma_start(out=xt_sb[:, c, :], in_=xt[:, c, :])

                with tc.tile_pool(name="mmp", bufs=6, space="PSUM") as mmp:
                    # ---- Q^T / K^T projections, weight-stationary: one
                    # LDWEIGHTS per (pair, chunk) feeds all 4 blocks.
                    for p in range(NPAIR):
                        qps = [
                            mmp.tile([128, 512], F32, tag="mm", name=f"qp{sb}")
                            for sb in range(NS)
                        ]
                        for c in range(ND):
                            for sb in range(NS):
                                nc.tensor.matmul(
                                    qps[sb][:],
                                    lhsT=wq_sb[:, p, c, :],
                                    rhs=xt_sb[:, c, sb * 512:(sb + 1) * 512],
                                    start=(c == 0),
                                    stop=(c == ND - 1),
                                )
                        for sb in range(NS):
                            nc.scalar.activation(
                                out=qt_sb[:, p, sb * 512:(sb + 1) * 512],
                                in_=qps[sb][:],
                                func=mybir.ActivationFunctionType.Identity,
                                bias=bq_sb[:, p:p + 1],
                            )
                        kps = [
                            mmp.tile([128, 512], F32, tag="mm", name=f"kp{sb}")
                            for sb in range(NS)
                        ]
                        for c in range(ND):
                            for sb in range(NS):
                                nc.tensor.matmul(
                                    kps[sb][:],
                                    lhsT=wk_sb[:, p, c, :],
                                    rhs=xt_sb[:, c, sb * 512:(sb + 1) * 512],
                                    start=(c == 0),
                                    stop=(c == ND - 1),
                                )
                        for sb in range(NS):
                            nc.scalar.activation(
                                out=kt_sb[:, p, sb * 512:(sb + 1) * 512],
                                in_=kps[sb][:],
                                func=mybir.ActivationFunctionType.Identity,
                                bias=bk_sb[:, p:p + 1],
                            )

                    # ---- V in natural layout [s, dk], 4 heads at once
                    for s16 in range(NS16):
                        vp = mmp.tile([128, 256], F32, tag="vp", name="vp", bufs=2)
                        for c in range(ND):
                            nc.tensor.matmul(
                                vp[:],
                                lhsT=xt_sb[:, c, s16 * 128:(s16 + 1) * 128],
                                rhs=wv_sb[:, c, :],
                                start=(c == 0),
                                stop=(c == ND - 1),
                            )
                        for p in range(NPAIR):
                            nc.vector.tensor_copy(
                                out=va_sb[:, p, s16, 0:64],
                                in_=vp[:, (2 * p) * 64:(2 * p) * 64 + 64],
                            )
                            nc.vector.tensor_copy(
                                out=va_sb[:, p, s16, 128:192],
                                in_=vp[:, (2 * p + 1) * 64:(2 * p + 1) * 64 + 64],
                            )

                # ---- attention + output projection, per query block
                with (
                    tc.tile_pool(name="stp", bufs=2, space="PSUM") as stp,
                    tc.tile_pool(name="ctxp", bufs=2, space="PSUM") as ctxp,
                    tc.tile_pool(name="auxp", bufs=1, space="PSUM") as auxp,
                    tc.tile_pool(name="ptp", bufs=3) as ptp,
                    tc.tile_pool(name="smp", bufs=2) as smp,
                    tc.tile_pool(name="outp", bufs=3) as outp,
                ):
                    def emit_norm(ctx_e, ctx_o, p, qb):
                        # per-pair normalization. Both denominators sit
                        # in the odd ctx bank (den_o at p0, den_e at
                        # p32), so ONE lane-local reciprocal_approx_fast
                        # at base partition 0 covers both; partitions
                        # 1:32 hold zeros -> garbage reciprocals that
                        # are never read.
                        r = smp.tile([33, 512], F32, tag="r", name="r")
                        nc.vector.reciprocal_approx_fast(
                            out=r[:], in_=ctx_o[0:33, :]
                        )
                        rb = smp.tile([33, 512], BF16, tag="rb", name="rb")
                        nc.vector.tensor_copy(out=rb[:], in_=r[:])
                        for j, ctx_ps in ((0, ctx_e), (1, ctx_o)):
                            even = j == 0
                            lo = 0 if even else 64
                            bc_ps = auxp.tile(
                                [128, 512], F32, tag="aux", name="bc_ps", bufs=2
                            )
                            nc.tensor.matmul(
                                bc_ps[:],
                                lhsT=(bcsel_sb[32:33, :] if even
                                      else bcsel_sb[0:1, :]),
                                rhs=rb[32:33, :] if even else rb[0:1, :],
                                start=True,
                                stop=True,
                            )
                            bc_sb = smp.tile(
                                [128, 512], F32, tag="bcs", name="bc_sb"
                            )
                            nc.vector.tensor_copy(
                                out=bc_sb[lo:lo + 64, :],
                                in_=bc_ps[lo:lo + 64, :],
                            )
                            nc.vector.tensor_mul(
                                out=ctxcat_sb[lo:lo + 64, p,
                                              qb * 512:(qb + 1) * 512],
                                in0=ctx_ps[lo:lo + 64, :],
                                in1=bc_sb[lo:lo + 64, :],
                            )

                    def emit_outproj_s16(s16):
                        ot = outp.tile([128, 2, 512], BF16, tag="ot", name="ot")
                        for do in range(2):
                            op = auxp.tile(
                                [128, 512], F32, tag="aux", name="op", bufs=2
                            )
                            nc.tensor.matmul(
                                op[:],
                                lhsT=ctxcat_sb[:, 0, s16 * 128:(s16 + 1) * 128],
                                rhs=wo_sb[:, 0, do * 512:(do + 1) * 512],
                                start=True,
                                stop=False,
                            )
                            nc.tensor.matmul(
                                op[:],
                                lhsT=ctxcat_sb[:, 1, s16 * 128:(s16 + 1) * 128],
                                rhs=wo_sb[:, 1, do * 512:(do + 1) * 512],
                                start=False,
                                stop=True,
                            )
                            nc.vector.tensor_copy(out=ot[:, do, :], in_=op[:])
                        nc.gpsimd.dma_start(
                            out=out[s16 * 128:(s16 + 1) * 128, :],
                            in_=ot[:],
                        )

                    oproj_q = []
                    pending = None
                    for qb in range(NS):
                        nch = (qb + 1) * 4
                        for p in range(NPAIR):
                            qs = qt_sb[:, p, qb * 512:(qb + 1) * 512]
                            ctx_e = ctxp.tile(
                                [128, 512], F32, tag="ctx", name="ctx_e"
                            )
                            ctx_o = ctxp.tile(
                                [128, 512], F32, tag="ctx", name="ctx_o"
                            )
                            pts = {}

                            def emit_scores(c):
                                diag = c >= qb * 4
                                f0 = 128 * (c - qb * 4) if diag else 0
                                st2 = stp.tile(
                                    [128, 2, 512], F32, tag="st", name="st2"
                                )
                                # row-tiled head pair: even on PE rows
                                # 0:64, odd on 64:128, concurrent.
                                nc.tensor.matmul(
                                    st2[:, 0, f0:512],
                                    lhsT=kt_sb[0:64, p,
                                               c * 128:(c + 1) * 128],
                                    rhs=qs[0:64, f0:512],
                                    start=True,
                                    stop=True,
                                )
                                nc.tensor.matmul(
                                    st2[:, 1, f0:512],
                                    lhsT=kt_sb[64:128, p,
                                               c * 128:(c + 1) * 128],
                                    rhs=qs[64:128, f0:512],
                                    start=True,
                                    stop=True,
                                )
                                pt2 = ptp.tile(
                                    [128, 2, 512], BF16, tag="pt", name="pt2"
                                )
                                nc.scalar.activation(
                                    out=pt2[:, :, f0:512],
                                    in_=st2[:, :, f0:512],
                                    func=mybir.ActivationFunctionType.Exp,
                                    scale=0.125,
                                )
                                if diag:
                                    # causal triangle: zero pt at q < kv
                                    for j in range(2):
                                        nc.vector.tensor_mul(
                                            out=pt2[:, j, f0:f0 + 128],
                                            in0=pt2[:, j, f0:f0 + 128],
                                            in1=mask01_sb[:],
                                        )
                                pts[c] = pt2

                            # software pipeline: scores/exp for chunks
                            # c+1/c+2 are emitted (and prioritized on
                            # the PE) BEFORE the exp-dependent ctx/den
                            # matmuls of chunk c and before the previous
                            # pair's norm/outproj tail, so ACT always
                            # has an exp ready to chew on.
                            emit_scores(0)
                            if nch > 1:
                                emit_scores(1)
                            if pending is not None:
                                emit_norm(*pending)
                                pending = None
                            for c in range(nch):
                                if c + 2 < nch:
                                    emit_scores(c + 2)
                                diag = c >= qb * 4
                                f0 = 128 * (c - qb * 4) if diag else 0
                                pt2 = pts.pop(c)
                                if c == 0:
                                    # establish bank B: full 128-part
                                    # write (zeros -> parts 0:64) with
                                    # the bank-clear, before any
                                    # accumulators.
                                    nc.tensor.matmul(
                                        ctx_o[:, f0:512] if diag else ctx_o[:],
                                        lhsT=va_sb[:, p, c, 64:192],
                                        rhs=pt2[:, 1, f0:512],
                                        start=True,
                                        stop=(nch == 1),
                                        skip_group_check=True,
                                    )
                                else:
                                    nc.tensor.matmul(
                                        ctx_o[64:128, f0:512] if diag
                                        else ctx_o[64:128, :],
                                        lhsT=va_sb[:, p, c, 128:192],
                                        rhs=pt2[:, 1, f0:512],
                                        start=False,
                                        stop=(c == nch - 1),
                                        skip_group_check=True,
                                    )
                                nc.tensor.matmul(
                                    ctx_o[0:1, f0:512] if diag else ctx_o[0:1, :],
                                    lhsT=onescol_sb[:, 0:1],
                                    rhs=pt2[:, 1, f0:512],
                                    start=False,
                                    stop=(c == nch - 1),
                                    skip_group_check=True,
                                )
                                nc.tensor.matmul(
                                    ctx_o[32:33, f0:512] if diag
                                    else ctx_o[32:33, :],
                                    lhsT=onescol_sb[:, 0:1],
                                    rhs=pt2[:, 0, f0:512],
                                    start=False,
                                    stop=(c == nch - 1),
                                    skip_group_check=True,
                                )
                                nc.tensor.matmul(
                                    ctx_e[0:64, f0:512] if diag
                                    else ctx_e[0:64, :],
                                    lhsT=va_sb[:, p, c, 0:64],
                                    rhs=pt2[:, 0, f0:512],
                                    start=(c == 0),
                                    stop=(c == nch - 1),
                                )
                                if oproj_q and c >= 1:
                                    emit_outproj_s16(oproj_q.pop(0))
                            pending = (ctx_e, ctx_o, p, qb)
                        if qb > 0:
                            oproj_q.extend(range((qb - 1) * 4, qb * 4))

                    emit_norm(*pending)
                    while oproj_q:
                        emit_outproj_s16(oproj_q.pop(0))
                    for s16 in range((NS - 1) * 4, NS * 4):
                        emit_outproj_s16(s16)
    if not nc.is_finalized():
        nc.finalize()
    return nc


def _prep_inputs(embeddings, Wq, bq, Wk, bk, Wv, bv, Wo, bo):
    import ml_dtypes
    bf16_t = ml_dtypes.bfloat16

    embeddings = np.asarray(embeddings, np.float32)
    Wq, bq = np.asarray(Wq, np.float32), np.asarray(bq, np.float32)
    Wk, bk = np.asarray(Wk, np.float32), np.asarray(bk, np.float32)
    Wv = np.asarray(Wv, np.float32)
    Wo = np.asarray(Wo, np.float32)

    p_idx = np.arange(128)
    # causal triangle within a diag block: valid (1.0) iff q_off >= kv_off
    mask01 = (np.arange(128)[None, :] >= p_idx[:, None]).astype(bf16_t)
    bcsel = np.zeros((128, 128), np.float32)
    bcsel[0, 64:128] = 1.0   # odd head: broadcast r[p0] to rows 64:128
    bcsel[32, 0:64] = 1.0    # even head: broadcast r[p32] to rows 0:64
    bcsel = bcsel.astype(bf16_t)
    onescol = np.ones((128, 8), np.float32).astype(bf16_t)

    in_maps = []
    for c in range(NCORES):
        b, g = c // 4, c % 4
        hs = HPC * g
        # xt: [128 (d within chunk), ND, S]
        xt = np.ascontiguousarray(
            embeddings[b].T.reshape(ND, 128, S).transpose(1, 0, 2)
        ).astype(bf16_t)
        # wq/wk: [128 (d within chunk), NPAIR, ND, 128 (pair dk)]
        wq2 = np.stack(
            [np.concatenate([Wq[hs + 2 * p], Wq[hs + 2 * p + 1]], axis=1)
             for p in range(NPAIR)]
        )  # [NPAIR, D, 128]
        wq2 = np.ascontiguousarray(
            wq2.reshape(NPAIR, ND, 128, 128).transpose(2, 0, 1, 3)
        ).astype(bf16_t)
        wk2 = np.stack(
            [np.concatenate([Wk[hs + 2 * p], Wk[hs + 2 * p + 1]], axis=1)
             for p in range(NPAIR)]
        )
        wk2 = np.ascontiguousarray(
            wk2.reshape(NPAIR, ND, 128, 128).transpose(2, 0, 1, 3)
        ).astype(bf16_t)
        # wv: [128 (d within chunk), ND, 256 (4 heads)]
        wv4 = np.concatenate([Wv[hs + h] for h in range(HPC)], axis=1)
        wv4 = np.ascontiguousarray(
            wv4.reshape(ND, 128, 256).transpose(1, 0, 2)
        ).astype(bf16_t)
        # wo: [128, 2, D]
        wo4 = np.ascontiguousarray(
            Wo[hs * DK:(hs + HPC) * DK, :].reshape(2, 128, D).transpose(1, 0, 2)
        ).astype(bf16_t)
        bq2 = np.stack(
            [np.concatenate([bq[hs + 2 * p], bq[hs + 2 * p + 1]])
             for p in range(NPAIR)], axis=1
        )
        bk2 = np.stack(
            [np.concatenate([bk[hs + 2 * p], bk[hs + 2 * p + 1]])
             for p in range(NPAIR)], axis=1
        )
        in_maps.append({
            "xt": np.ascontiguousarray(xt),
            "wq": np.ascontiguousarray(wq2),
            "wk": np.ascontiguousarray(wk2),
            "wv": np.ascontiguousarray(wv4),
            "wo": wo4,
            "bq": np.ascontiguousarray(bq2),
            "bk": np.ascontiguousarray(bk2),
            "mask01": mask01,
            "bcsel": bcsel,
            "onescol": onescol,
        })
    return in_maps


def kernel(embeddings, Wq, bq, Wk, bk, Wv, bv, Wo, bo, _trace=False, _trace_kw=None):
    if "nc" not in _CACHE:
        _CACHE["nc"] = _build_bass()
    nc = _CACHE["nc"]
    in_maps = _prep_inputs(embeddings, Wq, bq, Wk, bk, Wv, bv, Wo, bo)
    kw = dict(_trace_kw or {})
    res = run_bass_kernel_spmd(
        nc, in_maps, core_ids=list(range(NCORES)), trace=_trace, **kw
    )
    _CACHE["last_result"] = res
    bo32 = np.asarray(bo, np.float32)
    bv32 = np.asarray(bv, np.float32)
    Wo32 = np.asarray(Wo, np.float32)
    # softmax rows sum to 1, so the V bias contributes bv_cat @ Wo to
    # every output row; fold it into the output bias on the host.
    bo_eff = bo32 + bv32.reshape(-1) @ Wo32
    out = np.empty((B, S, D), np.float32)
    for b in range(B):
        acc = np.asarray(res.results[4 * b]["out"], np.float32)
        for g in range(1, 4):
            acc = acc + np.asarray(res.results[4 * b + g]["out"], np.float32)
        out[b] = acc + bo_eff
    return out


# revision 19
# speedup vs baseline: 1.0137x; 1.0137x over previous
"""Causal multi-head attention on 8 trn2 NeuronCores.

Sharding: core c handles batch b=c//4 and heads [4*(c%4), 4*(c%4)+4).
Each core computes its 4 heads' attention plus the partial output
projection against the matching 256 rows of Wo; the host sums the 4
partials per batch (the all-reduce implied by row-sharding Wo) and adds
bo_eff = bo + bv_cat @ Wo (the V-bias folds out of the kernel because
softmax rows sum to one).

v2 layout strategy (bf16 matmul operands, fp32 PSUM accumulation):
  - X^T [D,S] bf16 in SBUF; every projection contracts d on partitions.
  - Q^T/K^T per head-pair [128, S] bf16 (two heads stacked on
    partitions). Weight-stationary loop order: one LDWEIGHTS per
    (pair, d-chunk) feeds 4 query-block matmuls.
  - Scores transposed ST[kv, q] = K^T.T @ Q^T, head pair ROW-TILED:
    even head contracts on partitions 0:64 (tile_position (0,0)), odd
    on 64:128 ((64,0)) -> the two matmuls run concurrently in the PE
    array, writing adjacent PSUM banks of one [128, 2, 512] tile.
  - One exp per (pair, chunk) over both heads' scores (scale=1/8
    applied inside the activation); output pt bf16. Causal masking is
    a post-exp elementwise multiply with a 0/1 triangle on DVE -
    no mask matmuls, no -1e9 constants.
  - ctx^T via Vaug trick (ones column carries the softmax denominator
    into a spare PSUM partition). Normalization: reciprocal_approx_fast
    (~5x faster than DVE reciprocal), rank-1 PE broadcast, one DVE
    multiply straight into bf16 ctxcat. No bias add (folded into host).
  - Output projection bf16, PSUM->SBUF eviction on DVE, bf16 DMA out.
"""

import sys

for _p in ("/opt/trn_rl_repo", "/root/.axon_site/_ro/trn_rl_repo"):
    if _p not in sys.path:
        sys.path.insert(0, _p)

import numpy as np

import concourse.bass as bass
import concourse.bacc as bacc
import concourse.tile as tile
from concourse import mybir
from concourse.bass_utils import run_bass_kernel_spmd

F32 = mybir.dt.float32
BF16 = mybir.dt.bfloat16

B, S, D, H, DK = 2, 2048, 1024, 16, 64
NCORES = 8
HPC = 4          # heads per core
NPAIR = 2        # head pairs per core
ND = D // 128    # 8 contraction chunks over d
NS = S // 512    # 4 query blocks
NS16 = S // 128  # 16 sequence chunks

_CACHE = {}


def _build_bass():
    nc = bacc.Bacc(None)
    xt = nc.dram_tensor("xt", [128, ND, S], BF16, kind="ExternalInput")
    wq = nc.dram_tensor("wq", [128, NPAIR, ND, 128], BF16, kind="ExternalInput")
    wk = nc.dram_tensor("wk", [128, NPAIR, ND, 128], BF16, kind="ExternalInput")
    wv = nc.dram_tensor("wv", [128, ND, 256], BF16, kind="ExternalInput")
    wo = nc.dram_tensor("wo", [128, 2, D], BF16, kind="ExternalInput")
    bq = nc.dram_tensor("bq", [128, NPAIR], F32, kind="ExternalInput")
    bk = nc.dram_tensor("bk", [128, NPAIR], F32, kind="ExternalInput")
    mask01 = nc.dram_tensor("mask01", [128, 128], BF16, kind="ExternalInput")
    bcsel = nc.dram_tensor("bcsel", [128, 128], BF16, kind="ExternalInput")
    onescol = nc.dram_tensor("onescol", [128, 8], BF16, kind="ExternalInput")
    out = nc.dram_tensor("out", [S, D], BF16, kind="ExternalOutput")

    with nc.allow_low_precision("bf16 operands; accumulation stays fp32 in PSUM"), \
            tile.TileContext(nc) as tc:
        with (
            tc.tile_pool(name="consts", bufs=1) as consts,
            tc.tile_pool(name="qkv", bufs=1) as qkv,
        ):
            wq_sb = consts.tile([128, NPAIR, ND, 128], BF16, tag="wq")
            wk_sb = consts.tile([128, NPAIR, ND, 128], BF16, tag="wk")
            wv_sb = consts.tile([128, ND, 256], BF16, tag="wv")
            wo_sb = consts.tile([128, 2, D], BF16, tag="wo")
            bq_sb = consts.tile([128, NPAIR], F32, tag="bq")
            bk_sb = consts.tile([128, NPAIR], F32, tag="bk")
            mask01_sb = consts.tile([128, 128], BF16, tag="mask01")
            bcsel_sb = consts.tile([128, 128], BF16, tag="bcsel")
            onescol_sb = consts.tile([128, 8], BF16, tag="onescol")

            qt_sb = qkv.tile([128, NPAIR, S], BF16, tag="qt")
            kt_sb = qkv.tile([128, NPAIR, S], BF16, tag="kt")
            # V per pair: cols 0:64 V_even | 64:128 zeros | 128:192
            # V_odd. ctx_e uses cols 0:64 -> parts 0:64 of bank A.
            # ctx_o: at c==0, cols 64:192 (M=128, start=True) writes the
            # whole bank B (zeros to parts 0:64, ctx to 64:128) so the
            # accumulate bits are established for every partition; for
            # c>0 it uses cols 128:192 with col-tile (0,64). Softmax
            # denominators are rank-1 matmuls into bank B parts 0 (odd)
            # and 32 (even) at col groups 0/1, so for c>0 the ctx pair
            # and the denom pair each run concurrently in the PE array.
            va_sb = qkv.tile([128, NPAIR, NS16, 192], BF16, tag="va")
            ctxcat_sb = qkv.tile([128, 2, S], BF16, tag="ctxcat")

            # small consts first (tiny transfers, needed early)
            actwarm = consts.tile([1, 2], F32, tag="actwarm")
            nc.sync.dma_start(out=bq_sb[:], in_=bq[:])
            # preload the exp table set while DMAs stream (one-time
            # ACT_TABLE_LOAD ~2.7us otherwise lands mid-attention)
            nc.scalar.activation(
                out=actwarm[:], in_=bq_sb[0:1, 0:2],
                func=mybir.ActivationFunctionType.Exp,
            )
            nc.sync.dma_start(out=bk_sb[:], in_=bk[:])
            nc.sync.dma_start(out=mask01_sb[:], in_=mask01[:])
            nc.sync.dma_start(out=bcsel_sb[:], in_=bcsel[:])
            nc.sync.dma_start(out=onescol_sb[:], in_=onescol[:])
            for p in range(NPAIR):
                nc.vector.memset(va_sb[:, p, :, 64:128], 0.0)

            with (
                tc.tile_pool(name="xp", bufs=1) as xp,
            ):
                xt_sb = xp.tile([128, ND, S], BF16, tag="xt")
                nc.gpsimd.dma_start(out=wq_sb[:, 0], in_=wq[:, 0])
                nc.scalar.dma_start(out=wk_sb[:, 0], in_=wk[:, 0])
                nc.gpsimd.dma_start(out=wq_sb[:, 1], in_=wq[:, 1])
                nc.scalar.dma_start(out=wk_sb[:, 1], in_=wk[:, 1])
                nc.gpsimd.dma_start(out=wv_sb[:], in_=wv[:])
                nc.scalar.dma_start(out=wo_sb[:], in_=wo[:])
                for c in range(ND):
                    nc.sync.dma_start(out=xt_sb[:, c, :], in_=xt[:, c, :])

                with tc.tile_pool(name="mmp", bufs=6, space="PSUM") as mmp:
                    # ---- Q^T / K^T projections, weight-stationary: one
                    # LDWEIGHTS per (pair, chunk) feeds all 4 blocks.
                    for p in range(NPAIR):
                        qps = [
                            mmp.tile([128, 512], F32, tag="mm", name=f"qp{sb}")
                            for sb in range(NS)
                        ]
                        for c in range(ND):
                            for sb in range(NS):
                                nc.tensor.matmul(
                                    qps[sb][:],
                                    lhsT=wq_sb[:, p, c, :],
                                    rhs=xt_sb[:, c, sb * 512:(sb + 1) * 512],
                                    start=(c == 0),
                                    stop=(c == ND - 1),
                                )
                        for sb in range(NS):
                            nc.scalar.activation(
                                out=qt_sb[:, p, sb * 512:(sb + 1) * 512],
                                in_=qps[sb][:],
                                func=mybir.ActivationFunctionType.Identity,
                                bias=bq_sb[:, p:p + 1],
                            )
                        kps = [
                            mmp.tile([128, 512], F32, tag="mm", name=f"kp{sb}")
                            for sb in range(NS)
                        ]
                        for c in range(ND):
                            for sb in range(NS):
                                nc.tensor.matmul(
                                    kps[sb][:],
                                    lhsT=wk_sb[:, p, c, :],
                                    rhs=xt_sb[:, c, sb * 512:(sb + 1) * 512],
                                    start=(c == 0),
                                    stop=(c == ND - 1),
                                )
                        for sb in range(NS):
                            nc.scalar.activation(
                                out=kt_sb[:, p, sb * 512:(sb + 1) * 512],
                                in_=kps[sb][:],
                                func=mybir.ActivationFunctionType.Identity,
                                bias=bk_sb[:, p:p + 1],
                            )

                    # ---- V in natural layout [s, dk], 4 heads at once
                    for s16 in range(NS16):
                        vp = mmp.tile([128, 256], F32, tag="vp", name="vp", bufs=2)
                        for c in range(ND):
                            nc.tensor.matmul(
                                vp[:],
                                lhsT=xt_sb[:, c, s16 * 128:(s16 + 1) * 128],
                                rhs=wv_sb[:, c, :],
                                start=(c == 0),
                                stop=(c == ND - 1),
                            )
                        for p in range(NPAIR):
                            nc.vector.tensor_copy(
                                out=va_sb[:, p, s16, 0:64],
                                in_=vp[:, (2 * p) * 64:(2 * p) * 64 + 64],
                            )
                            nc.vector.tensor_copy(
                                out=va_sb[:, p, s16, 128:192],
                                in_=vp[:, (2 * p + 1) * 64:(2 * p + 1) * 64 + 64],
                            )

                # ---- attention + output projection, per query block
                with (
                    tc.tile_pool(name="stp", bufs=2, space="PSUM") as stp,
                    tc.tile_pool(name="ctxp", bufs=2, space="PSUM") as ctxp,
                    tc.tile_pool(name="auxp", bufs=1, space="PSUM") as auxp,
                    tc.tile_pool(name="ptp", bufs=3) as ptp,
                    tc.tile_pool(name="smp", bufs=2) as smp,
                    tc.tile_pool(name="outp", bufs=3) as outp,
                ):
                    def emit_norm(ctx_e, ctx_o, p, qb):
                        # per-pair normalization. Both denominators sit
                        # in the odd ctx bank (den_o at p0, den_e at
                        # p32), so ONE lane-local reciprocal_approx_fast
                        # at base partition 0 covers both; partitions
                        # 1:32 hold zeros -> garbage reciprocals that
                        # are never read.
                        r = smp.tile([33, 512], F32, tag="r", name="r")
                        nc.vector.reciprocal_approx_fast(
                            out=r[:], in_=ctx_o[0:33, :]
                        )
                        rb = smp.tile([33, 512], BF16, tag="rb", name="rb")
                        nc.vector.tensor_copy(out=rb[:], in_=r[:])
                        for j, ctx_ps in ((0, ctx_e), (1, ctx_o)):
                            even = j == 0
                            lo = 0 if even else 64
                            bc_ps = auxp.tile(
                                [128, 512], F32, tag="bc", name="bc_ps"
                            )
                            nc.tensor.matmul(
                                bc_ps[:],
                                lhsT=(bcsel_sb[32:33, :] if even
                                      else bcsel_sb[0:1, :]),
                                rhs=rb[32:33, :] if even else rb[0:1, :],
                                start=True,
                                stop=True,
                            )
                            bc_sb = smp.tile(
                                [128, 512], F32, tag="bcs", name="bc_sb"
                            )
                            nc.vector.tensor_copy(
                                out=bc_sb[lo:lo + 64, :],
                                in_=bc_ps[lo:lo + 64, :],
                            )
                            nc.vector.tensor_mul(
                                out=ctxcat_sb[lo:lo + 64, p,
                                              qb * 512:(qb + 1) * 512],
                                in0=ctx_ps[lo:lo + 64, :],
                                in1=bc_sb[lo:lo + 64, :],
                            )

                    def emit_outproj_s16(s16):
                        ot = outp.tile([128, 2, 512], BF16, tag="ot", name="ot")
                        for do in range(2):
                            op = auxp.tile(
                                [128, 512], F32, tag="op", name="op"
                            )
                            nc.tensor.matmul(
                                op[:],
                                lhsT=ctxcat_sb[:, 0, s16 * 128:(s16 + 1) * 128],
                                rhs=wo_sb[:, 0, do * 512:(do + 1) * 512],
                                start=True,
                                stop=False,
                            )
                            nc.tensor.matmul(
                                op[:],
                                lhsT=ctxcat_sb[:, 1, s16 * 128:(s16 + 1) * 128],
                                rhs=wo_sb[:, 1, do * 512:(do + 1) * 512],
                                start=False,
                                stop=True,
                            )
                            nc.vector.tensor_copy(out=ot[:, do, :], in_=op[:])
                        nc.gpsimd.dma_start(
                            out=out[s16 * 128:(s16 + 1) * 128, :],
                            in_=ot[:],
                        )

                    oproj_q = []
                    pending = None
                    for qb in range(NS):
                        nch = (qb + 1) * 4
                        for p in range(NPAIR):
                            qs = qt_sb[:, p, qb * 512:(qb + 1) * 512]
                            ctx_e = ctxp.tile(
                                [128, 512], F32, tag="ctx", name="ctx_e"
                            )
                            ctx_o = ctxp.tile(
                                [128, 512], F32, tag="ctx", name="ctx_o"
                            )
                            pts = {}

                            def emit_scores(c):
                                diag = c >= qb * 4
                                f0 = 128 * (c - qb * 4) if diag else 0
                                st2 = stp.tile(
                                    [128, 2, 512], F32, tag="st", name="st2"
                                )
                                # row-tiled head pair: even on PE rows
                                # 0:64, odd on 64:128, concurrent.
                                nc.tensor.matmul(
                                    st2[:, 0, f0:512],
                                    lhsT=kt_sb[0:64, p,
                                               c * 128:(c + 1) * 128],
                                    rhs=qs[0:64, f0:512],
                                    start=True,
                                    stop=True,
                                )
                                nc.tensor.matmul(
                                    st2[:, 1, f0:512],
                                    lhsT=kt_sb[64:128, p,
                                               c * 128:(c + 1) * 128],
                                    rhs=qs[64:128, f0:512],
                                    start=True,
                                    stop=True,
                                )
                                pt2 = ptp.tile(
                                    [128, 2, 512], BF16, tag="pt", name="pt2"
                                )
                                nc.scalar.activation(
                                    out=pt2[:, :, f0:512],
                                    in_=st2[:, :, f0:512],
                                    func=mybir.ActivationFunctionType.Exp,
                                    scale=0.125,
                                )
                                if diag:
                                    # causal triangle: zero pt at q < kv
                                    for j in range(2):
                                        nc.vector.tensor_mul(
                                            out=pt2[:, j, f0:f0 + 128],
                                            in0=pt2[:, j, f0:f0 + 128],
                                            in1=mask01_sb[:],
                                        )
                                pts[c] = pt2

                            # software pipeline: scores/exp for chunks
                            # c+1/c+2 are emitted (and prioritized on
                            # the PE) BEFORE the exp-dependent ctx/den
                            # matmuls of chunk c and before the previous
                            # pair's norm/outproj tail, so ACT always
                            # has an exp ready to chew on.
                            emit_scores(0)
                            if nch > 1:
                                emit_scores(1)
                            if pending is not None:
                                emit_norm(*pending)
                                pending = None
                            for c in range(nch):
                                if c + 2 < nch:
                                    emit_scores(c + 2)
                                diag = c >= qb * 4
                                f0 = 128 * (c - qb * 4) if diag else 0
                                pt2 = pts.pop(c)
                                if c == 0:
                                    # establish bank B: full 128-part
                                    # write (zeros -> parts 0:64) with
                                    # the bank-clear, before any
                                    # accumulators.
                                    nc.tensor.matmul(
                                        ctx_o[:, f0:512] if diag else ctx_o[:],
                                        lhsT=va_sb[:, p, c, 64:192],
                                        rhs=pt2[:, 1, f0:512],
                                        start=True,
                                        stop=(nch == 1),
                                        skip_group_check=True,
                                    )
                                else:
                                    nc.tensor.matmul(
                                        ctx_o[64:128, f0:512] if diag
                                        else ctx_o[64:128, :],
                                        lhsT=va_sb[:, p, c, 128:192],
                                        rhs=pt2[:, 1, f0:512],
                                        start=False,
                                        stop=(c == nch - 1),
                                        skip_group_check=True,
                                    )
                                nc.tensor.matmul(
                                    ctx_o[0:1, f0:512] if diag else ctx_o[0:1, :],
                                    lhsT=onescol_sb[:, 0:1],
                                    rhs=pt2[:, 1, f0:512],
                                    start=False,
                                    stop=(c == nch - 1),
                                    skip_group_check=True,
                                )
                                nc.tensor.matmul(
                                    ctx_o[32:33, f0:512] if diag
                                    else ctx_o[32:33, :],
                                    lhsT=onescol_sb[:, 0:1],
                                    rhs=pt2[:, 0, f0:512],
                                    start=False,
                                    stop=(c == nch - 1),
                                    skip_group_check=True,
                                )
                                nc.tensor.matmul(
                                    ctx_e[0:64, f0:512] if diag
                                    else ctx_e[0:64, :],
                                    lhsT=va_sb[:, p, c, 0:64],
                                    rhs=pt2[:, 0, f0:512],
                                    start=(c == 0),
                                    stop=(c == nch - 1),
                                )
                                if oproj_q and c >= 1:
                                    emit_outproj_s16(oproj_q.pop(0))
                            pending = (ctx_e, ctx_o, p, qb)
                        if qb > 0:
                            oproj_q.extend(range((qb - 1) * 4, qb * 4))

                    emit_norm(*pending)
                    while oproj_q:
                        emit_outproj_s16(oproj_q.pop(0))
                    for s16 in range((NS - 1) * 4, NS * 4):
                        emit_outproj_s16(s16)
    if not nc.is_finalized():
        nc.finalize()
    return nc


def _prep_inputs(embeddings, Wq, bq, Wk, bk, Wv, bv, Wo, bo):
    import ml_dtypes
    bf16_t = ml_dtypes.bfloat16

    embeddings = np.asarray(embeddings, np.float32)
    Wq, bq = np.asarray(Wq, np.float32), np.asarray(bq, np.float32)
    Wk, bk = np.asarray(Wk, np.float32), np.asarray(bk, np.float32)
    Wv = np.asarray(Wv, np.float32)
    Wo = np.asarray(Wo, np.float32)

    p_idx = np.arange(128)
    # causal triangle within a diag block: valid (1.0) iff q_off >= kv_off
    mask01 = (np.arange(128)[None, :] >= p_idx[:, None]).astype(bf16_t)
    bcsel = np.zeros((128, 128), np.float32)
    bcsel[0, 64:128] = 1.0   # odd head: broadcast r[p0] to rows 64:128
    bcsel[32, 0:64] = 1.0    # even head: broadcast r[p32] to rows 0:64
    bcsel = bcsel.astype(bf16_t)
    onescol = np.ones((128, 8), np.float32).astype(bf16_t)

    in_maps = []
    for c in range(NCORES):
        b, g = c // 4, c % 4
        hs = HPC * g
        # xt: [128 (d within chunk), ND, S]
        xt = np.ascontiguousarray(
            embeddings[b].T.reshape(ND, 128, S).transpose(1, 0, 2)
        ).astype(bf16_t)
        # wq/wk: [128 (d within chunk), NPAIR, ND, 128 (pair dk)]
        wq2 = np.stack(
            [np.concatenate([Wq[hs + 2 * p], Wq[hs + 2 * p + 1]], axis=1)
             for p in range(NPAIR)]
        )  # [NPAIR, D, 128]
        wq2 = np.ascontiguousarray(
            wq2.reshape(NPAIR, ND, 128, 128).transpose(2, 0, 1, 3)
        ).astype(bf16_t)
        wk2 = np.stack(
            [np.concatenate([Wk[hs + 2 * p], Wk[hs + 2 * p + 1]], axis=1)
             for p in range(NPAIR)]
        )
        wk2 = np.ascontiguousarray(
            wk2.reshape(NPAIR, ND, 128, 128).transpose(2, 0, 1, 3)
        ).astype(bf16_t)
        # wv: [128 (d within chunk), ND, 256 (4 heads)]
        wv4 = np.concatenate([Wv[hs + h] for h in range(HPC)], axis=1)
        wv4 = np.ascontiguousarray(
            wv4.reshape(ND, 128, 256).transpose(1, 0, 2)
        ).astype(bf16_t)
        # wo: [128, 2, D]
        wo4 = np.ascontiguousarray(
            Wo[hs * DK:(hs + HPC) * DK, :].reshape(2, 128, D).transpose(1, 0, 2)
        ).astype(bf16_t)
        bq2 = np.stack(
            [np.concatenate([bq[hs + 2 * p], bq[hs + 2 * p + 1]])
             for p in range(NPAIR)], axis=1
        )
        bk2 = np.stack(
            [np.concatenate([bk[hs + 2 * p], bk[hs + 2 * p + 1]])
             for p in range(NPAIR)], axis=1
        )
        in_maps.append({
            "xt": np.ascontiguousarray(xt),
            "wq": np.ascontiguousarray(wq2),
            "wk": np.ascontiguousarray(wk2),
            "wv": np.ascontiguousarray(wv4),
            "wo": wo4,
            "bq": np.ascontiguousarray(bq2),
            "bk": np.ascontiguousarray(bk2),
            "mask01": mask01,
            "bcsel": bcsel,
            "onescol": onescol,
        })
    return in_maps


def kernel(embeddings, Wq, bq, Wk, bk, Wv, bv, Wo, bo, _trace=False, _trace_kw=None):
    if "nc" not in _CACHE:
        _CACHE["nc"] = _build_bass()
    nc = _CACHE["nc"]
    in_maps = _prep_inputs(embeddings, Wq, bq, Wk, bk, Wv, bv, Wo, bo)
    kw = dict(_trace_kw or {})
    res = run_bass_kernel_spmd(
        nc, in_maps, core_ids=list(range(NCORES)), trace=_trace, **kw
    )
    _CACHE["last_result"] = res
    bo32 = np.asarray(bo, np.float32)
    bv32 = np.asarray(bv, np.float32)
    Wo32 = np.asarray(Wo, np.float32)
    # softmax rows sum to 1, so the V bias contributes bv_cat @ Wo to
    # every output row; fold it into the output bias on the host.
    bo_eff = bo32 + bv32.reshape(-1) @ Wo32
    out = np.empty((B, S, D), np.float32)
    for b in range(B):
        acc = np.asarray(res.results[4 * b]["out"], np.float32)
        for g in range(1, 4):
            acc = acc + np.asarray(res.results[4 * b + g]["out"], np.float32)
        out[b] = acc + bo_eff
    return out
